# revision 5
# baseline (speedup 1.0000x reference)
"""Trainium2 Bass kernel for nn_BuddingLayer (moe_routing).

Computation (B=512, SIN=SOUT=2048, K=128 buds):
  dense = (x * ~mask) @ weight.T + bias          mask = one-hot(sat_idx)
  per bud k (v = x[:, sat_idx[k]]):
    h1 = relu(v * c1[k] + b1[k])                 c1[k,j] = sum_i W1[k,i,j]/3
    h2 = relu(h1 @ W2[k] + b2[k])                [B, 3]
    u += relu(h2 @ W3[k] + b3[k])                [B, 2048]
  out = dense + u

Sharding: output-feature split, 256 columns per core (8 cores), compute in
transposed layout [o_part, b_free].  Host does slicing/transposition only;
all math (including masking, fp16 casts, c1 reduction) runs on device.

Bud path on device: one bud per 32-row PE group; super-tile t packs buds
4t..4t+3 at row-group bases {0,32,64,96}.  K=4 matmul per (bud, o-chunk)
with a constant-1.0 4th rhs row whose lhsT row carries b3 (bias for free).
PSUM fp32 -> relu exits (ScalarE activation + VectorE tensor_scalar_max)
-> fp16 -> block tree-sums (VectorE + GpSimd) -> + dense -> DMA out.
"""

import numpy as np

N_CORES = 8
B = 512
SIN = 2048
SOUT = 2048
K = 128
OC = SOUT // N_CORES          # 256 output cols per core
NCHUNK = SIN // 128           # 16 contraction chunks for dense
NT = K // 4                   # 32 super-tiles
BLK = 16                      # buds per tree block
NBLK = K // BLK               # 8 blocks per o-chunk

# exit/add engine split (tuned): fraction of exit-units on ScalarE, and
# number of tree blocks (of 16 per oc-pair... 16 total) handled by GpSimd
ACT_EXIT_FRAC = 0.80
GP_BLOCKS = 5

_compiled = {}


def _build(chunk_status):
    """Build the SPMD Bass program.  chunk_status: tuple of 'full'|'partial'|'clean'
    per 128-row input chunk ('full' = entirely masked, skip)."""
    import concourse.bacc as bacc
    import concourse.mybir as mybir
    import concourse.tile as tile

    f32, f16 = mybir.dt.float32, mybir.dt.float16
    AL = mybir.AluOpType
    AF = mybir.ActivationFunctionType

    nc = bacc.Bacc("TRN2", target_bir_lowering=False, debug=False,
                   num_devices=N_CORES)

    # ---- DRAM I/O (per core) ----
    xT = nc.dram_tensor("xT", [SIN, B], f32, kind="ExternalInput")
    xsatT = nc.dram_tensor("xsatT", [K, B], f32, kind="ExternalInput")
    maskT = nc.dram_tensor("maskT", [SIN, 1], f32, kind="ExternalInput")
    wT = nc.dram_tensor("wT", [SIN, OC], f32, kind="ExternalInput")
    biasc = nc.dram_tensor("biasc", [1, OC], f32, kind="ExternalInput")
    w1d = nc.dram_tensor("w1d", [K, 9], f32, kind="ExternalInput")
    b1d = nc.dram_tensor("b1d", [K, 3], f32, kind="ExternalInput")
    w2d = nc.dram_tensor("w2d", [K, 9], f32, kind="ExternalInput")
    b2d = nc.dram_tensor("b2d", [K, 3], f32, kind="ExternalInput")
    w3d = nc.dram_tensor("w3d", [K, 3 * OC], f32, kind="ExternalInput")
    b3d = nc.dram_tensor("b3d", [K, OC], f32, kind="ExternalInput")
    outT = nc.dram_tensor("outT", [OC, B], f32, kind="ExternalOutput")

    with tile.TileContext(nc) as tc:
        with (
            tc.tile_pool(name="const", bufs=1) as cp,
            tc.tile_pool(name="stage", bufs=2) as stp,
            tc.tile_pool(name="rblk", bufs=2) as rbp,
            tc.tile_pool(name="psum", bufs=3, space="PSUM") as pp,
            tc.tile_pool(name="psumd", bufs=2, space="PSUM") as ppd,
        ):
            # ---------- tiny constants ----------
            w1s = cp.tile([K, 9], f32)
            nc.sync.dma_start(w1s[:], w1d.ap())
            b1s = cp.tile([K, 3], f32)
            nc.sync.dma_start(b1s[:], b1d.ap())
            w2s = cp.tile([K, 9], f32)
            nc.sync.dma_start(w2s[:], w2d.ap())
            b2s = cp.tile([K, 3], f32)
            nc.sync.dma_start(b2s[:], b2d.ap())
            masks = cp.tile([128, NCHUNK], f32)
            nc.sync.dma_start(masks[:], maskT.ap().rearrange("(c p) one -> p (c one)", p=128))
            bias_sb = cp.tile([1, OC], f16)
            bias_f32 = stp.tile([1, OC], f32, tag="bias_st")
            nc.sync.dma_start(bias_f32[:], biasc.ap())
            nc.vector.tensor_copy(bias_sb[:], bias_f32[:])

            # c1[k, j] = (W1[k,0,j] + W1[k,1,j] + W1[k,2,j]) / 3
            c1a = cp.tile([K, 3], f32)
            nc.vector.tensor_tensor(c1a[:], w1s[:, 0:3], w1s[:, 3:6], AL.add)
            c1 = cp.tile([K, 3], f32)
            nc.vector.tensor_tensor(c1[:], c1a[:], w1s[:, 6:9], AL.add)
            c1s = cp.tile([K, 3], f32)
            nc.vector.tensor_scalar_mul(c1s[:], c1[:], 1.0 / 3.0)

            # ---------- h path: v -> h1 -> h2 (layout [k, b]) ----------
            v = cp.tile([K, B], f32)
            nc.sync.dma_start(v[:], xsatT.ap())
            h1 = [cp.tile([K, B], f16, tag=f"h1_{j}", name=f"h1_{j}") for j in range(3)]
            for j in range(3):
                nc.scalar.activation(h1[j][:], v[:], AF.Relu,
                                     bias=b1s[:, j:j + 1], scale=c1s[:, j:j + 1])
            h2 = [cp.tile([K, B], f16, tag=f"h2_{j}", name=f"h2_{j}") for j in range(3)]
            for j in range(3):
                m = [stp.tile([K, B], f16, tag=f"hm{i}", name=f"hm{i}_{j}") for i in range(3)]
                for i in range(3):
                    nc.vector.tensor_scalar_mul(m[i][:], h1[i][:], w2s[:, 3 * i + j : 3 * i + j + 1])
                s0 = stp.tile([K, B], f16, tag="hs0")
                nc.vector.tensor_tensor(s0[:], m[0][:], m[1][:], AL.add)
                s1 = stp.tile([K, B], f16, tag="hs1")
                nc.vector.tensor_tensor(s1[:], s0[:], m[2][:], AL.add)
                nc.scalar.activation(h2[j][:], s1[:], AF.Relu, bias=b2s[:, j:j + 1])

            # ---------- W3B / b3 -> packed fp16, H2T packed fp16 ----------
            w3s32 = stp.tile([K, 3 * OC], f32, tag="w3st")
            nc.sync.dma_start(w3s32[:], w3d.ap())
            w3s = cp.tile([K, 3 * OC], f16)
            nc.vector.tensor_copy(w3s[:], w3s32[:])
            b3s32 = stp.tile([K, OC], f32, tag="b3st")
            nc.sync.dma_start(b3s32[:], b3d.ap())
            b3s = cp.tile([K, OC], f16)
            nc.vector.tensor_copy(b3s[:], b3s32[:])

            h2t = cp.tile([128, 512 * NT], f16)       # [128, 16384]
            nc.vector.memset(h2t[:], 1.0)
            w3b = cp.tile([128, OC * NT], f16)        # [128, 8192]
            for g in range(4):
                for j in range(3):
                    nc.sync.dma_start(
                        h2t[32 * g + j : 32 * g + j + 1, :].rearrange("p (t b) -> p t b", b=512),
                        h2[j][g::4, :],
                    )
                    nc.sync.dma_start(
                        w3b[32 * g + j : 32 * g + j + 1, :].rearrange("p (t o) -> p t o", o=OC),
                        w3s[g::4, OC * j : OC * (j + 1)],
                    )
                nc.sync.dma_start(
                    w3b[32 * g + 3 : 32 * g + 4, :].rearrange("p (t o) -> p t o", o=OC),
                    b3s[g::4, :],
                )

            # ---------- x / w load + cast (+mask) to fp16 ----------
            x16 = cp.tile([128, 512 * NCHUNK], f16)   # 16 chunks x [128,512]
            w16 = cp.tile([128, OC * NCHUNK], f16)    # 16 chunks x [128,256]
            live = [c for c in range(NCHUNK) if chunk_status[c] != "full"]
            for c in live:
                xst = stp.tile([128, B], f32, tag="xst")
                nc.sync.dma_start(xst[:], xT.ap()[128 * c : 128 * (c + 1), :])
                dst = x16[:, 512 * c : 512 * (c + 1)]
                if chunk_status[c] == "partial":
                    nc.vector.tensor_scalar_mul(dst, xst[:], masks[:, c : c + 1])
                else:
                    nc.vector.tensor_copy(dst, xst[:])
                wst = stp.tile([128, OC], f32, tag="wst")
                nc.sync.dma_start(wst[:], wT.ap()[128 * c : 128 * (c + 1), :])
                nc.vector.tensor_copy(w16[:, OC * c : OC * (c + 1)], wst[:])

            # ---------- dense matmuls (+bias row via dedicated ones tile) ----------
            ones_row = cp.tile([1, 512], f16)
            nc.vector.memset(ones_row[:], 1.0)
            dense_sb = cp.tile([128, 1024], f32)      # [dense_oc0 | dense_oc1]
            for oc in range(2):
                dps = ppd.tile([128, 512], f32, tag="dense_ps")
                first = True
                for c in live:
                    nc.tensor.matmul(
                        dps[:],
                        w16[:, OC * c + 128 * oc : OC * c + 128 * oc + 128],
                        x16[:, 512 * c : 512 * (c + 1)],
                        start=first, stop=False,
                    )
                    first = False
                # bias row: lhsT = bias_sb[0:1, oc slice], rhs = ones row of h2t
                nc.tensor.matmul(
                    dps[:], bias_sb[:, 128 * oc : 128 * (oc + 1)],
                    ones_row[:], start=first, stop=True,
                )
                nc.vector.tensor_copy(dense_sb[:, 512 * oc : 512 * (oc + 1)], dps[:])

            # ---------- bud matmuls + relu exits + block trees ----------
            blocksums = cp.tile([128, 512 * NBLK * 2], f16)   # [oc, blk]
            vscr = cp.tile([128, 4096 + 2048 + 1024], f16)
            gscr = cp.tile([128, 4096 + 2048 + 1024], f16)
            fscr = cp.tile([128, 2048 + 1024], f16)
            fin = cp.tile([128, 1024], f32)
            outsb = cp.tile([128, 1024], f32)
            gp_set = set(range(0, 2 * NBLK, max(1, (2 * NBLK) // max(GP_BLOCKS, 1)))[:GP_BLOCKS]) if GP_BLOCKS else set()
            blk_idx = 0
            for oc in range(2):
                for blk in range(NBLK):
                    rb = rbp.tile([128, 512 * BLK], f16, tag="rblk")
                    for u in range(BLK // 2):         # exit-units: 2 buds each
                        k0 = BLK * blk + 2 * u
                        zps = pp.tile([128, 1024], f32, tag="zps")
                        for d in range(2):
                            k = k0 + d
                            t, g = divmod(k, 4)
                            nc.tensor.matmul(
                                zps[:, 512 * d : 512 * (d + 1)],
                                w3b[32 * g : 32 * g + 4, OC * t + 128 * oc : OC * t + 128 * oc + 128],
                                h2t[32 * g : 32 * g + 4, 512 * t : 512 * (t + 1)],
                                start=True, stop=True, tile_position=(32 * g, 0),
                            )
                        dst = rb[:, 1024 * u : 1024 * (u + 1)]
                        if (u % 5) < round(5 * ACT_EXIT_FRAC):
                            nc.scalar.activation(dst, zps[:], AF.Relu)
                        else:
                            nc.vector.tensor_scalar_max(dst, zps[:], 0.0)
                    # tree-sum the 16 buds of this block -> blocksums col
                    use_gp = blk_idx in gp_set
                    eng = nc.gpsimd if use_gp else nc.vector
                    scr = gscr if use_gp else vscr
                    src, width, off = rb[:], 512 * BLK, 0
                    while width > 1024:
                        half = width // 2
                        dst_ = scr[:, off : off + half]
                        eng.tensor_tensor(dst_, src[:, 0:half], src[:, half:width], AL.add)
                        src, width, off = scr[:, off : off + half], half, off + half
                    eng.tensor_tensor(
                        blocksums[:, 512 * blk_idx : 512 * (blk_idx + 1)],
                        src[:, 0:512], src[:, 512:1024], AL.add)
                    blk_idx += 1

            # ---------- final: sum blocksums per oc, add dense, store ----------
            for oc in range(2):
                base = 512 * NBLK * oc
                cur, width, off = blocksums[:, base : base + 512 * NBLK], 512 * NBLK, 0
                while width > 1024:
                    half = width // 2
                    nc.vector.tensor_tensor(fscr[:, off : off + half], cur[:, 0:half],
                                            cur[:, half:width], AL.add)
                    cur, width, off = fscr[:, off : off + half], half, off + half
                ft = fin[:, 512 * oc : 512 * (oc + 1)]
                nc.vector.tensor_tensor(ft, cur[:, 0:512], cur[:, 512:1024], AL.add)
                ot = outsb[:, 512 * oc : 512 * (oc + 1)]
                nc.vector.tensor_tensor(ot, dense_sb[:, 512 * oc : 512 * (oc + 1)], ft, AL.add)
                nc.sync.dma_start(outT.ap()[128 * oc : 128 * (oc + 1), :], ot)

    nc.finalize()
    return nc


def _prep_inputs(x, sat_idx, weight, bias, W1, b1, W2, b2, W3, b3):
    """Host-side shard/layout prep. Returns (chunk_status, per-core input maps)."""
    x = np.ascontiguousarray(np.asarray(x, np.float32))
    sat = np.asarray(sat_idx).astype(np.int64)
    weight = np.asarray(weight, np.float32)
    bias = np.asarray(bias, np.float32)

    mask = np.ones(SIN, np.float32)
    mask[sat] = 0.0
    chunk_status = []
    for c in range(NCHUNK):
        mc = mask[128 * c : 128 * (c + 1)]
        if not mc.any():
            chunk_status.append("full")
        elif mc.all():
            chunk_status.append("clean")
        else:
            chunk_status.append("partial")
    chunk_status = tuple(chunk_status)

    xT = np.ascontiguousarray(x.T)                       # [SIN, B]
    xsatT = np.ascontiguousarray(x[:, sat].T)            # [K, B]
    maskT = np.ascontiguousarray(mask[:, None])          # [SIN, 1]
    w1h = np.ascontiguousarray(np.asarray(W1, np.float32).reshape(K, 9))
    w2h = np.ascontiguousarray(np.asarray(W2, np.float32).reshape(K, 9))
    b1h = np.ascontiguousarray(np.asarray(b1, np.float32))
    b2h = np.ascontiguousarray(np.asarray(b2, np.float32))
    W3 = np.asarray(W3, np.float32)
    b3 = np.asarray(b3, np.float32)

    in_maps = []
    for c in range(N_CORES):
        sl = slice(OC * c, OC * (c + 1))
        in_maps.append({
            "xT": xT,
            "xsatT": xsatT,
            "maskT": maskT,
            "wT": np.ascontiguousarray(weight[sl, :].T),          # [SIN, OC]
            "biasc": np.ascontiguousarray(bias[sl][None, :]),     # [1, OC]
            "w1d": w1h, "b1d": b1h, "w2d": w2h, "b2d": b2h,
            "w3d": np.ascontiguousarray(W3[:, :, sl].reshape(K, 3 * OC)),
            "b3d": np.ascontiguousarray(b3[:, sl]),
        })
    return chunk_status, in_maps


def kernel(**inputs) -> np.ndarray:
    from concourse.bass_utils import run_bass_kernel_spmd

    chunk_status, in_maps = _prep_inputs(
        inputs["x"], inputs["sat_idx"], inputs["weight"], inputs["bias"],
        inputs["W1"], inputs["b1"], inputs["W2"], inputs["b2"],
        inputs["W3"], inputs["b3"],
    )
    if chunk_status not in _compiled:
        _compiled[chunk_status] = _build(chunk_status)
    nc = _compiled[chunk_status]
    res = run_bass_kernel_spmd(nc, in_maps, core_ids=list(range(N_CORES)))
    outT = np.concatenate([res.results[c]["outT"] for c in range(N_CORES)], axis=0)
    return np.ascontiguousarray(outT.T).astype(np.float32)


# revision 17
# speedup vs baseline: 5907.9289x; 5907.9289x over previous
"""Trainium2 Bass kernel for nn_BuddingLayer (moe_routing).

Computation (B=512, SIN=SOUT=2048, K=128 buds):
  dense = (x * ~mask) @ weight.T + bias          mask = one-hot(sat_idx)
  per bud k (v = x[:, sat_idx[k]]):
    h1 = relu(v * c1[k] + b1[k])                 c1[k,j] = sum_i W1[k,i,j]/3
    h2 = relu(h1 @ W2[k] + b2[k])                [B, 3]
    u += relu(h2 @ W3[k] + b3[k])                [B, 2048]
  out = dense + u

Sharding: output-feature split, 256 columns per core (8 cores), compute in
transposed layout [o_part, b_free].  Host does slicing/transposition only;
all math (masking, fp16 casts, c1 reduction) runs on device.

Bud path: one bud per 32-row PE group; super-tile t packs buds 4t..4t+3 at
row-group bases {0,32,64,96}.  K=4 matmul per (bud, o-chunk) with a
constant-1.0 4th rhs row whose lhsT row carries b3 (bias folded into the MM).
PSUM fp32 -> relu exits (ScalarE activation / VectorE tensor_scalar_max)
-> fp16 -> block tree-sums (VectorE + GpSimd tail blocks) -> + dense -> out.
"""

import numpy as np

N_CORES = 8
B = 512
SIN = 2048
SOUT = 2048
K = 128
OC = SOUT // N_CORES          # 256 output cols per core
NCHUNK = SIN // 128           # 16 contraction chunks for dense
NT = K // 4                   # 32 super-tiles
BLK = 16                      # buds per tree block
NBLK = K // BLK               # 8 blocks per o-chunk

# tuning knobs
ACT_UNITS_OF_8 = 6            # of every 8 exit-units, this many go to ScalarE
GP_SUM_BLOCKS = 3             # tree blocks (of 16) summed on GpSimd (rest VectorE)

_compiled = {}


def _build(chunk_status, repeat=1):
    """Build the SPMD Bass program.  chunk_status: tuple of 'full'|'partial'|'clean'
    per 128-row input chunk ('full' = entirely masked, skip).  repeat>1 emits
    the whole body multiple times (benchmarking only)."""
    import concourse.bacc as bacc
    import concourse.mybir as mybir
    import concourse.tile as tile

    f32, f16 = mybir.dt.float32, mybir.dt.float16
    AL = mybir.AluOpType
    AF = mybir.ActivationFunctionType

    nc = bacc.Bacc("TRN2", target_bir_lowering=False, debug=False,
                   num_devices=N_CORES)

    # ---- DRAM I/O (per core) ----
    xT = nc.dram_tensor("xT", [SIN, B], f32, kind="ExternalInput")
    xsatT = nc.dram_tensor("xsatT", [K, B], f32, kind="ExternalInput")
    maskT = nc.dram_tensor("maskT", [SIN, 1], f32, kind="ExternalInput")
    wT = nc.dram_tensor("wT", [SIN, OC], f32, kind="ExternalInput")
    biasc = nc.dram_tensor("biasc", [1, OC], f32, kind="ExternalInput")
    w1d = nc.dram_tensor("w1d", [K, 9], f32, kind="ExternalInput")
    b1d = nc.dram_tensor("b1d", [K, 3], f32, kind="ExternalInput")
    w2d = nc.dram_tensor("w2d", [K, 9], f32, kind="ExternalInput")
    b2d = nc.dram_tensor("b2d", [K, 3], f32, kind="ExternalInput")
    w3d = nc.dram_tensor("w3d", [K, 3 * OC], f32, kind="ExternalInput")
    b3d = nc.dram_tensor("b3d", [K, OC], f32, kind="ExternalInput")
    outT = nc.dram_tensor("outT", [OC, B], f32, kind="ExternalOutput")

    with tile.TileContext(nc) as tc:
      for _rep in range(repeat):
        with (
            tc.tile_pool(name="const", bufs=1) as cp,
            tc.tile_pool(name="stage", bufs=2) as stp,
            tc.tile_pool(name="rblk", bufs=2) as rbp,
            tc.tile_pool(name="psum", bufs=3, space="PSUM") as pp,
            tc.tile_pool(name="psumd", bufs=1, space="PSUM") as ppd,
        ):
            # ---------- tiny constants ----------
            w1s = cp.tile([K, 9], f32)
            nc.sync.dma_start(w1s[:], w1d.ap())
            b1s = cp.tile([K, 3], f32)
            nc.sync.dma_start(b1s[:], b1d.ap())
            w2s = cp.tile([K, 9], f32)
            nc.sync.dma_start(w2s[:], w2d.ap())
            b2s = cp.tile([K, 3], f32)
            nc.sync.dma_start(b2s[:], b2d.ap())
            masks = cp.tile([128, NCHUNK], f32)
            nc.sync.dma_start(masks[:], maskT.ap().rearrange("(c p) one -> p (c one)", p=128))
            bias_sb = cp.tile([1, OC], f16)
            nc.gpsimd.dma_start(bias_sb[:], biasc.ap())

            # c1[k, j] = (W1[k,0,j] + W1[k,1,j] + W1[k,2,j]) / 3
            c1a = cp.tile([K, 3], f32)
            nc.vector.tensor_tensor(c1a[:], w1s[:, 0:3], w1s[:, 3:6], AL.add)
            c1 = cp.tile([K, 3], f32)
            nc.vector.tensor_tensor(c1[:], c1a[:], w1s[:, 6:9], AL.add)
            c1s = cp.tile([K, 3], f32)
            nc.vector.tensor_scalar_mul(c1s[:], c1[:], 1.0 / 3.0)

            # ---------- h path: v -> h1 -> h2 (layout [k, b]) ----------
            v = cp.tile([K, B], f32)
            nc.sync.dma_start(v[:], xsatT.ap())
            h1 = [cp.tile([K, B], f16, tag=f"h1_{j}", name=f"h1_{j}") for j in range(3)]
            for j in range(3):
                nc.scalar.activation(h1[j][:], v[:], AF.Relu,
                                     bias=b1s[:, j:j + 1], scale=c1s[:, j:j + 1])
            h2 = [cp.tile([K, B], f16, tag=f"h2_{j}", name=f"h2_{j}") for j in range(3)]
            for j in range(3):
                ma = stp.tile([K, B], f16, tag="hm0", name=f"hma{j}")
                nc.vector.tensor_scalar_mul(ma[:], h1[0][:], w2s[:, j : j + 1])
                mb = stp.tile([K, B], f16, tag="hm1", name=f"hmb{j}")
                nc.vector.tensor_scalar_mul(mb[:], h1[1][:], w2s[:, 3 + j : 4 + j])
                sab = stp.tile([K, B], f16, tag="hm0", name=f"hsab{j}")
                nc.vector.tensor_tensor(sab[:], ma[:], mb[:], AL.add)
                mc = stp.tile([K, B], f16, tag="hm1", name=f"hmc{j}")
                nc.vector.tensor_scalar_mul(mc[:], h1[2][:], w2s[:, 6 + j : 7 + j])
                s = stp.tile([K, B], f16, tag="hm0", name=f"hs{j}")
                nc.vector.tensor_tensor(s[:], sab[:], mc[:], AL.add)
                nc.scalar.activation(h2[j][:], s[:], AF.Relu, bias=b2s[:, j:j + 1])

            # ---------- W3B / b3 -> packed fp16, H2T packed fp16 ----------
            w3s = cp.tile([K, 3 * OC], f16)
            nc.gpsimd.dma_start(w3s[:], w3d.ap())
            b3s = cp.tile([K, OC], f16)
            nc.gpsimd.dma_start(b3s[:], b3d.ap())
            ones32 = cp.tile([32, 512], f16)
            nc.vector.memset(ones32[:], 1.0)

            h2t = cp.tile([128, 512 * NT], f16)       # [128, 16384]
            w3b = cp.tile([128, OC * NT], f16)        # [128, 8192]
            for g in range(4):
                for j in range(3):
                    nc.sync.dma_start(
                        h2t[32 * g + j : 32 * g + j + 1, :].rearrange("p (t b) -> p t b", b=512),
                        h2[j][g::4, :],
                    )
                    nc.sync.dma_start(
                        w3b[32 * g + j : 32 * g + j + 1, :].rearrange("p (t o) -> p t o", o=OC),
                        w3s[g::4, OC * j : OC * (j + 1)],
                    )
                nc.sync.dma_start(
                    h2t[32 * g + 3 : 32 * g + 4, :].rearrange("p (t b) -> p t b", b=512),
                    ones32[:],
                )
                nc.sync.dma_start(
                    w3b[32 * g + 3 : 32 * g + 4, :].rearrange("p (t o) -> p t o", o=OC),
                    b3s[g::4, :],
                )

            # ---------- dense inputs: one cast-DMA each for x and w ----------
            live = [c for c in range(NCHUNK) if chunk_status[c] != "full"]
            x16a = cp.tile([128, 512 * NCHUNK], f16)
            nc.gpsimd.dma_start(
                x16a[:].rearrange("p (c b) -> p c b", b=B),
                xT.ap().rearrange("(c p) b -> p c b", p=128))
            w16a = cp.tile([128, OC * NCHUNK], f16)
            nc.gpsimd.dma_start(
                w16a[:].rearrange("p (c o) -> p c o", o=OC),
                wT.ap().rearrange("(c p) o -> p c o", p=128))
            dps = ppd.tile([128, 1024], f32, name="dps")  # [:, :512]=oc0, [:, 512:]=oc1
            dense_state = {"first": True}

            def emit_dense_chunk(c):
                x16 = x16a[:, 512 * c : 512 * (c + 1)]
                if chunk_status[c] == "partial":
                    xm = stp.tile([128, B], f16, tag="x16m", name=f"x16m_{c}_{_rep}")
                    nc.vector.tensor_scalar_mul(xm[:], x16, masks[:, c : c + 1])
                    x16 = xm[:]
                for oc in range(2):
                    nc.tensor.matmul(dps[:, 512 * oc : 512 * (oc + 1)],
                                     w16a[:, OC * c + 128 * oc : OC * c + 128 * oc + 128],
                                     x16,
                                     start=dense_state["first"], stop=False)
                dense_state["first"] = False

            pending = list(live)
            for c in pending[:2]:
                emit_dense_chunk(c)
            pending = pending[2:]

            # ---------- bud matmuls + relu exits + block trees ----------
            blocksums = cp.tile([128, 512 * NBLK * 2], f16)   # [oc, blk]
            vscr = cp.tile([128, 4096 + 2048 + 1024 + 2048], f16)
            gscr = cp.tile([128, 4096 + 2048 + 1024], f16)
            outsb = cp.tile([128, 1024], f32)
            unit_counter = [0]
            n_blocks = 2 * NBLK
            modes = ["dve"] * n_blocks
            for i in range(GP_SUM_BLOCKS):
                modes[(i * n_blocks) // max(GP_SUM_BLOCKS, 1) % n_blocks] = "gp"
            blk_idx = 0
            for oc in range(2):
                for blk in range(NBLK):
                    rb = rbp.tile([128, 512 * BLK], f16, tag="rblk", name=f"rb{oc}_{blk}")
                    for u in range(BLK // 2):         # 2-bud exit units
                        k0 = BLK * blk + 2 * u
                        t = k0 // 4
                        g0 = k0 % 4                   # buds k0, k0+1 -> groups g0, g0+1
                        zps = pp.tile([128, 1024], f32, tag="zps", name=f"z{oc}_{blk}_{u}")
                        for d in range(2):
                            g = g0 + d
                            nc.tensor.matmul(
                                zps[:, 512 * d : 512 * (d + 1)],
                                w3b[32 * g : 32 * g + 4, OC * t + 128 * oc : OC * t + 128 * oc + 128],
                                h2t[32 * g : 32 * g + 4, 512 * t : 512 * (t + 1)],
                                start=True, stop=True, tile_position=(32 * g, 0),
                            )
                        dst = rb[:, 1024 * u : 1024 * (u + 1)]
                        if (u % 8) < ACT_UNITS_OF_8:
                            nc.scalar.activation(dst, zps[:], AF.Relu)
                        else:
                            nc.vector.tensor_scalar_max(dst, zps[:], 0.0)
                        unit_counter[0] += 1
                        if unit_counter[0] % 8 == 0 and pending:
                            emit_dense_chunk(pending.pop(0))
                    # tree-sum the 16 buds of this block -> blocksums col
                    bs = blocksums[:, 512 * blk_idx : 512 * (blk_idx + 1)]
                    mode = modes[blk_idx]
                    eng = nc.gpsimd if mode == "gp" else nc.vector
                    scr = gscr if mode == "gp" else vscr
                    src, width, off = rb[:], 512 * BLK, 0
                    while width > 1024:
                        half = width // 2
                        dst_ = scr[:, off : off + half]
                        eng.tensor_tensor(dst_, src[:, 0:half], src[:, half:width], AL.add)
                        src, width, off = scr[:, off : off + half], half, off + half
                    eng.tensor_tensor(bs, src[:, 0:512], src[:, 512:1024], AL.add)
                    blk_idx += 1

            for c in pending:
                emit_dense_chunk(c)
            for oc in range(2):   # bias row
                nc.tensor.matmul(dps[:, 512 * oc : 512 * (oc + 1)],
                                 bias_sb[:, 128 * oc : 128 * (oc + 1)],
                                 ones32[0:1, :], start=False, stop=True)
            dense_sb = cp.tile([128, 1024], f32)
            nc.vector.tensor_copy(dense_sb[:], dps[:])

            # ---------- final: sum blocksums per oc, add dense, store ----------
            for oc in range(2):
                base = 512 * NBLK * oc
                cur, width, off = blocksums[:, base : base + 512 * NBLK], 512 * NBLK, 0
                while width > 1024:
                    half = width // 2
                    nc.vector.tensor_tensor(vscr[:, off : off + half], cur[:, 0:half],
                                            cur[:, half:width], AL.add)
                    cur, width, off = vscr[:, off : off + half], half, off + half
                ft = vscr[:, 7168 + 512 * oc : 7168 + 512 * (oc + 1)]
                nc.vector.tensor_tensor(ft, cur[:, 0:512], cur[:, 512:1024], AL.add)
                ot = outsb[:, 512 * oc : 512 * (oc + 1)]
                nc.vector.tensor_tensor(ot, dense_sb[:, 512 * oc : 512 * (oc + 1)], ft, AL.add)
                nc.sync.dma_start(outT.ap()[128 * oc : 128 * (oc + 1), :], ot)

    nc.finalize()
    return nc


def _prep_inputs(x, sat_idx, weight, bias, W1, b1, W2, b2, W3, b3):
    """Host-side shard/layout prep. Returns (chunk_status, per-core input maps)."""
    x = np.ascontiguousarray(np.asarray(x, np.float32))
    sat = np.asarray(sat_idx).astype(np.int64)
    weight = np.asarray(weight, np.float32)
    bias = np.asarray(bias, np.float32)

    mask = np.ones(SIN, np.float32)
    mask[sat] = 0.0
    chunk_status = []
    for c in range(NCHUNK):
        mc = mask[128 * c : 128 * (c + 1)]
        if not mc.any():
            chunk_status.append("full")
        elif mc.all():
            chunk_status.append("clean")
        else:
            chunk_status.append("partial")
    chunk_status = tuple(chunk_status)

    xT = np.ascontiguousarray(x.T)                       # [SIN, B]
    xsatT = np.ascontiguousarray(x[:, sat].T)            # [K, B]
    maskT = np.ascontiguousarray(mask[:, None])          # [SIN, 1]
    w1h = np.ascontiguousarray(np.asarray(W1, np.float32).reshape(K, 9))
    w2h = np.ascontiguousarray(np.asarray(W2, np.float32).reshape(K, 9))
    b1h = np.ascontiguousarray(np.asarray(b1, np.float32))
    b2h = np.ascontiguousarray(np.asarray(b2, np.float32))
    W3 = np.asarray(W3, np.float32)
    b3 = np.asarray(b3, np.float32)

    in_maps = []
    for c in range(N_CORES):
        sl = slice(OC * c, OC * (c + 1))
        in_maps.append({
            "xT": xT,
            "xsatT": xsatT,
            "maskT": maskT,
            "wT": np.ascontiguousarray(weight[sl, :].T),          # [SIN, OC]
            "biasc": np.ascontiguousarray(bias[sl][None, :]),     # [1, OC]
            "w1d": w1h, "b1d": b1h, "w2d": w2h, "b2d": b2h,
            "w3d": np.ascontiguousarray(W3[:, :, sl].reshape(K, 3 * OC)),
            "b3d": np.ascontiguousarray(b3[:, sl]),
        })
    return chunk_status, in_maps


def kernel(**inputs) -> np.ndarray:
    from concourse.bass_utils import run_bass_kernel_spmd

    chunk_status, in_maps = _prep_inputs(
        inputs["x"], inputs["sat_idx"], inputs["weight"], inputs["bias"],
        inputs["W1"], inputs["b1"], inputs["W2"], inputs["b2"],
        inputs["W3"], inputs["b3"],
    )
    if chunk_status not in _compiled:
        _compiled[chunk_status] = _build(chunk_status)
    nc = _compiled[chunk_status]
    res = run_bass_kernel_spmd(nc, in_maps, core_ids=list(range(N_CORES)))
    outT = np.concatenate([res.results[c]["outT"] for c in range(N_CORES)], axis=0)
    return np.ascontiguousarray(outT.T).astype(np.float32)
